# revision 1
# baseline (speedup 1.0000x reference)
"""Multi-head attention (B=4, S=2048, D=1024, H=16) on 8 trn2 NeuronCores.

Sharding: batch (4-way) x head-half (2-way).  Core c = 2*b + hh handles batch b
and heads hh*8 .. hh*8+7.  Each core:
  1. QT/KT projections in d-on-partitions layout, V in tokens-on-partitions
     layout with a ones-column per head (softmax denominator comes out of the
     attn@V matmul as row 64).  All matmuls run as float32r (full PE rate,
     fp32-matmul numerics).
  2. Attention is processed per (q-half, head, key-tile), software-pipelined:
     scores_T = K_h @ Q_h^T into a 2-bank PSUM tile, one wide exp on the
     scalar engine -> attn_T slice (streamed flash-style, no SxS
     materialization), attn@V accumulates out_T per head in PSUM; scores for
     kt+1 are emitted before attn@V(kt) so the PE streams through exp latency.
  3. Tail per head: copy out of PSUM (frees banks fast), denominator row
     replicated across partitions with two 32-lane stream shuffles, fast
     Newton reciprocal, multiply into out_T.  The output projection consumes
     out_T directly as lhsT per q-half.  Host sums the two half-head partials
     and adds the bias.
"""

import sys

if "/opt/trn_rl_repo" not in sys.path:
    sys.path.insert(0, "/opt/trn_rl_repo")

import numpy as np

B, S, D = 4, 2048, 1024
H, HD = 16, 64
P = 128
DK = D // P            # 8 contraction chunks for the projections
NKT = S // P           # 16 token tiles
QB = 512
NQB = S // QB          # 4 q blocks
HB = 2 * QB            # q-half width (1024)
DH = 512               # head dims per core (8 heads)
NDC = DH // P          # 4 dout chunks per core
NHC = 8                # heads per core
VW = HD + 1            # V columns per head incl. the ones column
NCORES = 8

_PROG = [None]


def _build():
    import concourse.mybir as mybir
    import concourse.tile as tile
    from concourse import bacc

    f32 = mybir.dt.float32
    f32r = mybir.dt.float32r
    Exp = mybir.ActivationFunctionType.Exp

    nc = bacc.Bacc("TRN2", target_bir_lowering=False, debug=False)
    xq = nc.dram_tensor("xq", [D, S], f32r, kind="ExternalInput").ap()
    xk = nc.dram_tensor("xk", [D, S], f32r, kind="ExternalInput").ap()
    xv = nc.dram_tensor("xv", [D, S], f32r, kind="ExternalInput").ap()
    wq = nc.dram_tensor("wq", [D, DH], f32r, kind="ExternalInput").ap()
    wk = nc.dram_tensor("wk", [D, DH], f32r, kind="ExternalInput").ap()
    wv = nc.dram_tensor("wv", [D, DH], f32r, kind="ExternalInput").ap()
    wo = nc.dram_tensor("wo", [DH, D], f32r, kind="ExternalInput").ap()
    part = nc.dram_tensor("part", [S, D], f32, kind="ExternalOutput").ap()

    xq_v = xq.rearrange("(c p) s -> p c s", p=P)
    xk_v = xk.rearrange("(c p) s -> p c s", p=P)
    xv_v = xv.rearrange("(c p) s -> p c s", p=P)

    with tile.TileContext(nc) as tc:
        with tc.tile_pool(name="big", bufs=1) as big, tc.tile_pool(name="wp", bufs=2) as wp:
            QT = big.tile([P, NDC, S], f32r, tag="QT")
            KT = big.tile([P, NDC, S], f32r, tag="KT")
            V = big.tile([P, NKT, NHC * VW], f32r, tag="V")
            outT = big.tile([P, NDC, S], f32r, tag="outT")

            # ---- projections ------------------------------------------------
            with (
                tc.tile_pool(name="xc", bufs=12) as xc,
                tc.tile_pool(name="pp", bufs=4, space="PSUM") as pp,
            ):

                def proj_T(x_view, w_dram, out_t):
                    w_t = wp.tile([P, DK, DH], f32r, tag="w")
                    w_v = w_dram.rearrange("(c p) m -> p c m", p=P)
                    for dk in range(DK):
                        nc.sync.dma_start(w_t[:, dk], w_v[:, dk])
                    for qb in range(NQB):
                        xts = []
                        for dk in range(DK):
                            xt = xc.tile([P, QB], f32r, tag="xc")
                            nc.sync.dma_start(
                                xt[:], x_view[:, dk, qb * QB : (qb + 1) * QB]
                            )
                            xts.append(xt)
                        pts = [pp.tile([P, QB], f32, tag="pp", name=f"pp{i}") for i in range(NDC)]
                        for dk in range(DK):
                            for dc in range(NDC):
                                nc.tensor.matmul(
                                    pts[dc][:],
                                    w_t[:, dk, dc * P : (dc + 1) * P],
                                    xts[dk][:],
                                    start=(dk == 0),
                                    stop=(dk == DK - 1),
                                )
                        for dc in range(NDC):
                            dst = out_t[:, dc, qb * QB : (qb + 1) * QB]
                            if dc % 2 == 0:
                                nc.vector.tensor_copy(dst, pts[dc][:])
                            else:
                                nc.scalar.copy(dst, pts[dc][:])

                proj_T(xq_v, wq, QT)
                proj_T(xk_v, wk, KT)

                # V projection (tokens-on-partitions) + ones columns
                nc.vector.memset(V[:].bitcast(f32), 1.0)
                wv_t = wp.tile([P, DK, DH], f32r, tag="w")
                wv_v = wv.rearrange("(c p) m -> p c m", p=P)
                for dk in range(DK):
                    nc.sync.dma_start(wv_t[:, dk], wv_v[:, dk])
                for qb in range(NQB):
                    xts = []
                    for dk in range(DK):
                        xt = xc.tile([P, QB], f32r, tag="xc")
                        nc.sync.dma_start(
                            xt[:], xv_v[:, dk, qb * QB : (qb + 1) * QB]
                        )
                        xts.append(xt)
                    pts = [pp.tile([P, DH], f32, tag="pp", name=f"ppv{i}") for i in range(QB // P)]
                    for dk in range(DK):
                        for kt_in in range(QB // P):
                            nc.tensor.matmul(
                                pts[kt_in][:],
                                xts[dk][:, kt_in * P : (kt_in + 1) * P],
                                wv_t[:, dk, :],
                                start=(dk == 0),
                                stop=(dk == DK - 1),
                            )
                    for kt_in in range(QB // P):
                        kt = qb * (QB // P) + kt_in
                        nc.vector.tensor_copy(
                            V[:, kt].rearrange("p (h c) -> p h c", c=VW)[
                                :, :, 0:HD
                            ],
                            pts[kt_in][:].rearrange("p (h c) -> p h c", c=HD),
                        )

            # wo loads while attention runs (slot frees when wv_t is done)
            wo_t = wp.tile([P, NDC, D], f32r, tag="w")
            nc.sync.dma_start(wo_t[:], wo.rearrange("(c p) m -> p c m", p=P))

            # ---- attention + output projection, per q-half ------------------
            with (
                tc.tile_pool(name="attn", bufs=3) as attnp,
                tc.tile_pool(name="tail", bufs=1) as tailp,
                tc.tile_pool(name="asbp", bufs=2) as asbp,
                tc.tile_pool(name="stage", bufs=2) as stage,
                tc.tile_pool(name="acc", bufs=1, space="PSUM") as accp,
                tc.tile_pool(name="sc", bufs=2, space="PSUM") as scp,
                tc.tile_pool(name="po", bufs=2, space="PSUM") as pop,
            ):
                for half in range(2):
                    c0 = half * HB

                    def emit_scores(kt, hc, r0, c0=c0):
                        sct = scp.tile([P, HB], f32, tag="sc")
                        for j in range(2):
                            nc.tensor.matmul(
                                sct[:, j * QB : (j + 1) * QB],
                                KT[r0 : r0 + 64, hc, kt * P : (kt + 1) * P],
                                QT[
                                    r0 : r0 + 64,
                                    hc,
                                    c0 + j * QB : c0 + (j + 1) * QB,
                                ],
                                start=True,
                                stop=True,
                            )
                        return sct

                    def emit_attnv(acc, h, kt, at_t):
                        for j in range(2):
                            nc.tensor.matmul(
                                acc[0:VW, j * QB : (j + 1) * QB],
                                V[:, kt, h * VW : (h + 1) * VW],
                                at_t[:, j * QB : (j + 1) * QB],
                                start=(kt == 0),
                                stop=(kt == NKT - 1),
                            )

                    def emit_tail(acc, hp, hc):
                        # evacuate psum, replicate denominator, divide
                        asb = asbp.tile([96, HB], f32, tag="asb")
                        nc.vector.tensor_copy(asb[0:VW, :], acc[0:VW, :])
                        bc = tailp.tile([64, HB], f32, tag="bc")
                        nc.vector.stream_shuffle(
                            bc[0:32, :], asb[64:96, :], [0] * 32
                        )
                        nc.vector.stream_shuffle(
                            bc[32:64, :], asb[64:96, :], [0] * 32
                        )
                        rec = tailp.tile([64, HB], f32, tag="rec")
                        scr = tailp.tile([64, HB], f32, tag="scr")
                        nc.vector.reciprocal_approx_accurate(
                            rec[:], bc[:], scr[:]
                        )
                        if hp == 0:
                            nc.vector.tensor_mul(
                                outT[0:64, hc, c0 : c0 + HB],
                                asb[0:HD, :],
                                rec[:],
                            )
                        else:
                            tmp = tailp.tile([64, HB], f32r, tag="tmp")
                            nc.vector.tensor_mul(tmp[:], asb[0:HD, :], rec[:])
                            nc.sync.dma_start(
                                outT[64:128, hc, c0 : c0 + HB], tmp[:]
                            )

                    for h in range(NHC):
                        hp, hc = h % 2, h // 2
                        r0 = 64 * hp
                        acc = accp.tile([P, HB], f32, tag="acc")
                        sc_cur = emit_scores(0, hc, r0)
                        for kt in range(NKT):
                            at_t = attnp.tile([P, HB], f32r, tag="attn")
                            nc.scalar.activation(at_t[:], sc_cur[:], Exp)
                            if kt + 1 < NKT:
                                sc_cur = emit_scores(kt + 1, hc, r0)
                            emit_attnv(acc, h, kt, at_t)
                        emit_tail(acc, hp, hc)

                    # output projection for this q-half
                    for qt in range(half * (NKT // 2), (half + 1) * (NKT // 2)):
                        for do in range(2):
                            po = pop.tile([P, QB], f32, tag="po")
                            for dc in range(NDC):
                                nc.tensor.matmul(
                                    po[:],
                                    outT[:, dc, qt * P : (qt + 1) * P],
                                    wo_t[:, dc, do * QB : (do + 1) * QB],
                                    start=(dc == 0),
                                    stop=(dc == NDC - 1),
                                )
                            st = stage.tile([P, QB], f32, tag="st")
                            nc.vector.tensor_copy(st[:], po[:])
                            nc.sync.dma_start(
                                part[
                                    qt * P : (qt + 1) * P,
                                    do * QB : (do + 1) * QB,
                                ],
                                st[:],
                            )

    nc.compile()
    return nc


def _get_prog():
    if _PROG[0] is None:
        _PROG[0] = _build()
    return _PROG[0]


def make_in_maps(query, key, value, Wq, Wk, Wv, Wo):
    scale = np.float32(1.0 / np.sqrt(D))
    Wq_s = (np.asarray(Wq, np.float32) * scale).astype(np.float32)
    Wk_s = np.ascontiguousarray(np.asarray(Wk, np.float32))
    Wv_s = np.ascontiguousarray(np.asarray(Wv, np.float32))
    Wo_s = np.ascontiguousarray(np.asarray(Wo, np.float32))
    in_maps = []
    for b in range(B):
        xqT = np.ascontiguousarray(np.asarray(query[b], np.float32).T)
        xkT = np.ascontiguousarray(np.asarray(key[b], np.float32).T)
        xvT = np.ascontiguousarray(np.asarray(value[b], np.float32).T)
        for hh in range(2):
            sl = slice(hh * DH, (hh + 1) * DH)
            in_maps.append(
                {
                    "xq": xqT,
                    "xk": xkT,
                    "xv": xvT,
                    "wq": np.ascontiguousarray(Wq_s[:, sl]),
                    "wk": np.ascontiguousarray(Wk_s[:, sl]),
                    "wv": np.ascontiguousarray(Wv_s[:, sl]),
                    "wo": np.ascontiguousarray(Wo_s[sl, :]),
                }
            )
    return in_maps


def run(in_maps, trace=False, **kw):
    from concourse.bass_utils import run_bass_kernel_spmd

    nc = _get_prog()
    return run_bass_kernel_spmd(
        nc, in_maps, core_ids=list(range(NCORES)), trace=trace, **kw
    )


def kernel(query, key, value, Wq, Wk, Wv, Wo, bo):
    in_maps = make_in_maps(query, key, value, Wq, Wk, Wv, Wo)
    res = run(in_maps)
    bo = np.asarray(bo, np.float32)
    out = np.empty((B, S, D), np.float32)
    for b in range(B):
        out[b] = res.results[2 * b]["part"] + res.results[2 * b + 1]["part"] + bo
    return out



# revision 6
# speedup vs baseline: 1.4174x; 1.4174x over previous
"""Multi-head attention (B=4, S=2048, D=1024, H=16) on 8 trn2 NeuronCores.

Sharding: batch (4-way) x head-half (2-way).  Core c = 2*b + hh handles batch b
and heads hh*8 .. hh*8+7.  v2 of the kernel: all matmul operands in bf16
(halves DMA, enables fast weight load), restructured so the PE never idles
long enough for the HAM clock-gate to re-throttle it to 1.2 GHz (which cost
the fp32 baseline ~300us):

  1. KT proj (full), QT proj (q-half 0 only), V proj (full) run back-to-back
     on the PE; x/w DMA in bf16 double-buffered underneath.
  2. Attention runs per (q-half, head, key-tile): scores KQ^T into PSUM fp32,
     one exp per key-tile on the scalar engine -> bf16 attn tile, attn@V
     accumulates per head in PSUM (ones-column gives the softmax denominator).
     The scalar engine is the bottleneck of this phase (~1.1us/tile); the PE
     gap in each slot is filled by injecting leftover projection work: q-half-1
     QT projection during half-0 attention, half-0 output projection during
     half-1 attention.  Tail per head: denominator broadcast via two stream
     shuffles + Newton reciprocal on the vector engine.
  3. Half-1 output projection drains at the end.  Host sums the two half-head
     partials and adds the bias.
"""

import sys

if "/opt/trn_rl_repo" not in sys.path:
    sys.path.insert(0, "/opt/trn_rl_repo")

import numpy as np

B, S, D = 4, 2048, 1024
H, HD = 16, 64
P = 128
DK = D // P            # 8 contraction chunks for the projections
NKT = S // P           # 16 token tiles
QB = 512
DH = 512               # head dims per core (8 heads)
NDC = DH // P          # 4 dout chunks per core
NHC = 8                # heads per core
VW = HD + 1            # V columns per head incl. the ones column
HB = 1024              # q-half width
NCORES = 8

_PROG = [None]


def _build():
    import concourse.mybir as mybir
    import concourse.tile as tile
    from concourse import bacc

    f32 = mybir.dt.float32
    bf16 = mybir.dt.bfloat16
    Exp = mybir.ActivationFunctionType.Exp

    nc = bacc.Bacc("TRN2", target_bir_lowering=False, debug=False)
    xq = nc.dram_tensor("xq", [D, S], bf16, kind="ExternalInput").ap()
    xk = nc.dram_tensor("xk", [D, S], bf16, kind="ExternalInput").ap()
    xv = nc.dram_tensor("xv", [D, S], bf16, kind="ExternalInput").ap()
    wq = nc.dram_tensor("wq", [D, DH], bf16, kind="ExternalInput").ap()
    wk = nc.dram_tensor("wk", [D, DH], bf16, kind="ExternalInput").ap()
    wv = nc.dram_tensor("wv", [D, DH], bf16, kind="ExternalInput").ap()
    wo = nc.dram_tensor("wo", [DH, D], bf16, kind="ExternalInput").ap()
    part = nc.dram_tensor("part", [S, D], f32, kind="ExternalOutput").ap()

    xq_v = xq.rearrange("(c p) s -> p c s", p=P)
    xk_v = xk.rearrange("(c p) s -> p c s", p=P)
    xv_v = xv.rearrange("(c p) s -> p c s", p=P)

    with tile.TileContext(nc) as tc:
        with tc.tile_pool(name="big", bufs=1) as big, tc.tile_pool(name="wp", bufs=4) as wp:
            QT = big.tile([P, NDC, S], bf16, tag="QT")
            KT = big.tile([P, NDC, S], bf16, tag="KT")
            V = big.tile([P, NKT, NHC * VW], bf16, tag="V")
            outT = big.tile([P, NDC, S], bf16, tag="outT")

            # weight tiles live for the whole kernel (wq is reused by the
            # q-half-1 filler projection during attention)
            wk_t = wp.tile([P, DK, DH], bf16, tag="w", name="wk")
            wq_t = wp.tile([P, DK, DH], bf16, tag="w", name="wq")
            wv_t = wp.tile([P, DK, DH], bf16, tag="w", name="wv")
            wo_t = wp.tile([P, NDC, D], bf16, tag="w", name="wo")

            # ---- pre-attention projections -------------------------------
            with (
                tc.tile_pool(name="xc", bufs=16) as xc,
                tc.tile_pool(name="pp", bufs=4, space="PSUM") as pp,
            ):
                def load_w(w_t, w_dram):
                    w_v = w_dram.rearrange("(c p) m -> p c m", p=P)
                    for dk in range(DK):
                        nc.sync.dma_start(w_t[:, dk], w_v[:, dk])

                load_w(wk_t, wk)

                def proj_T(x_view, w_t, out_t, qbs):
                    for qb in qbs:
                        xts = []
                        for dk in range(DK):
                            xt = xc.tile([P, QB], bf16, tag="xc")
                            nc.sync.dma_start(
                                xt[:], x_view[:, dk, qb * QB : (qb + 1) * QB]
                            )
                            xts.append(xt)
                        pts = [pp.tile([P, QB], f32, tag="pp", name=f"pp{i}") for i in range(NDC)]
                        for dk in range(DK):
                            for dc in range(NDC):
                                nc.tensor.matmul(
                                    pts[dc][:],
                                    w_t[:, dk, dc * P : (dc + 1) * P],
                                    xts[dk][:],
                                    start=(dk == 0),
                                    stop=(dk == DK - 1),
                                )
                        for dc in range(NDC):
                            dst = out_t[:, dc, qb * QB : (qb + 1) * QB]
                            if dc % 2 == 0:
                                nc.vector.tensor_copy(dst, pts[dc][:])
                            else:
                                nc.scalar.copy(dst, pts[dc][:])

                proj_T(xk_v, wk_t, KT, [0])
                load_w(wq_t, wq)
                proj_T(xk_v, wk_t, KT, [1, 2, 3])
                proj_T(xq_v, wq_t, QT, [0])
                load_w(wv_t, wv)
                nc.sync.dma_start(wo_t[:], wo.rearrange("(c p) m -> p c m", p=P))
                proj_T(xq_v, wq_t, QT, [1])

                # V projection (tokens-on-partitions) + ones columns
                nc.vector.memset(V[:], 1.0)
                for qb in range(4):
                    xts = []
                    for dk in range(DK):
                        xt = xc.tile([P, QB], bf16, tag="xc")
                        nc.sync.dma_start(
                            xt[:], xv_v[:, dk, qb * QB : (qb + 1) * QB]
                        )
                        xts.append(xt)
                    pts = [pp.tile([P, DH], f32, tag="pp", name=f"ppv{i}") for i in range(QB // P)]
                    for dk in range(DK):
                        for kt_in in range(QB // P):
                            nc.tensor.matmul(
                                pts[kt_in][:],
                                xts[dk][:, kt_in * P : (kt_in + 1) * P],
                                wv_t[:, dk, :],
                                start=(dk == 0),
                                stop=(dk == DK - 1),
                            )
                    for kt_in in range(QB // P):
                        kt = qb * (QB // P) + kt_in
                        nc.vector.tensor_copy(
                            V[:, kt].rearrange("p (h c) -> p h c", c=VW)[
                                :, :, 0:HD
                            ],
                            pts[kt_in][:].rearrange("p (h c) -> p h c", c=HD),
                        )

            # ---- attention + fillers, per q-half --------------------------
            with (
                tc.tile_pool(name="attn", bufs=3) as attnp,
                tc.tile_pool(name="tail", bufs=1) as tailp,
                tc.tile_pool(name="asbp", bufs=2) as asbp,
                tc.tile_pool(name="stage", bufs=2) as stage,
                tc.tile_pool(name="xf", bufs=16) as xf,
                tc.tile_pool(name="acc", bufs=1, space="PSUM") as accp,
                tc.tile_pool(name="sc", bufs=2, space="PSUM") as scp,
                tc.tile_pool(name="fillpp", bufs=2, space="PSUM") as fillp,
            ):
                # -- filler micro-step generators --
                def qt23_steps():
                    """Project QT for q-half 1 (qb 2,3), in ~0.5us steps."""
                    xts_all = {}
                    cur = {}

                    def dma_qb(qb):
                        def go():
                            xts = []
                            for dk in range(DK):
                                xt = xf.tile([P, QB], bf16, tag="xf", name=f"xf{qb}_{dk}")
                                nc.sync.dma_start(
                                    xt[:], xq_v[:, dk, qb * QB : (qb + 1) * QB]
                                )
                                xts.append(xt)
                            xts_all[qb] = xts
                        return go

                    def mm_step(qb, dc, dk0):
                        def go():
                            if dk0 == 0:
                                cur[(qb, dc)] = fillp.tile([P, QB], f32, tag="fp", name=f"fq{qb}_{dc}")
                            pt = cur[(qb, dc)]
                            for dk in (dk0, dk0 + 1):
                                nc.tensor.matmul(
                                    pt[:],
                                    wq_t[:, dk, dc * P : (dc + 1) * P],
                                    xts_all[qb][dk][:],
                                    start=(dk == 0),
                                    stop=(dk == DK - 1),
                                )
                        return go

                    def copy_step(qb, dc):
                        def go():
                            nc.vector.tensor_copy(
                                QT[:, dc, qb * QB : (qb + 1) * QB],
                                cur[(qb, dc)][:],
                            )
                        return go

                    yield dma_qb(2)
                    yield dma_qb(3)
                    for qb in (2, 3):
                        for dc in range(NDC):
                            for dk0 in range(0, DK, 2):
                                yield mm_step(qb, dc, dk0)
                            yield copy_step(qb, dc)

                def oproj_steps(qts):
                    """Output projection for token tiles qts, in ~0.5us steps."""
                    cur = {}

                    def mm_step(qt, do, dc0):
                        def go():
                            if dc0 == 0:
                                cur[(qt, do)] = fillp.tile([P, QB], f32, tag="fp", name=f"fo{qt}_{do}")
                            po = cur[(qt, do)]
                            for dc in (dc0, dc0 + 1):
                                nc.tensor.matmul(
                                    po[:],
                                    outT[:, dc, qt * P : (qt + 1) * P],
                                    wo_t[:, dc, do * QB : (do + 1) * QB],
                                    start=(dc == 0),
                                    stop=(dc == NDC - 1),
                                )
                        return go

                    def out_step(qt, do):
                        def go():
                            st = stage.tile([P, QB], f32, tag="st", name=f"st{qt}_{do}")
                            nc.vector.tensor_copy(st[:], cur[(qt, do)][:])
                            nc.sync.dma_start(
                                part[
                                    qt * P : (qt + 1) * P,
                                    do * QB : (do + 1) * QB,
                                ],
                                st[:],
                            )
                        return go

                    for qt in qts:
                        for do in range(2):
                            for dc0 in range(0, NDC, 2):
                                yield mm_step(qt, do, dc0)
                            yield out_step(qt, do)

                def drain(it):
                    for s in it:
                        s()

                for half in range(2):
                    c0 = half * HB
                    fill_iter = qt23_steps() if half == 0 else oproj_steps(range(8))

                    def emit_scores(kt, hc, r0, c0=c0):
                        sct = scp.tile([P, HB], f32, tag="sc")
                        for j in range(2):
                            nc.tensor.matmul(
                                sct[:, j * QB : (j + 1) * QB],
                                KT[r0 : r0 + 64, hc, kt * P : (kt + 1) * P],
                                QT[
                                    r0 : r0 + 64,
                                    hc,
                                    c0 + j * QB : c0 + (j + 1) * QB,
                                ],
                                start=True,
                                stop=True,
                            )
                        return sct

                    def emit_attnv(acc, h, kt, at_t):
                        for j in range(2):
                            nc.tensor.matmul(
                                acc[0:VW, j * QB : (j + 1) * QB],
                                V[:, kt, h * VW : (h + 1) * VW],
                                at_t[:, j * QB : (j + 1) * QB],
                                start=(kt == 0),
                                stop=(kt == NKT - 1),
                            )

                    def emit_tail(acc, hp, hc, c0=c0):
                        # evacuate psum, replicate denominator, divide
                        asb = asbp.tile([96, HB], f32, tag="asb")
                        nc.vector.tensor_copy(asb[0:VW, :], acc[0:VW, :])
                        bc = tailp.tile([64, HB], f32, tag="bc")
                        nc.vector.stream_shuffle(
                            bc[0:32, :], asb[64:96, :], [0] * 32
                        )
                        nc.vector.stream_shuffle(
                            bc[32:64, :], asb[64:96, :], [0] * 32
                        )
                        rec = tailp.tile([64, HB], f32, tag="rec")
                        scr = tailp.tile([64, HB], f32, tag="scr")
                        nc.vector.reciprocal_approx_accurate(
                            rec[:], bc[:], scr[:]
                        )
                        if hp == 0:
                            nc.vector.tensor_mul(
                                outT[0:64, hc, c0 : c0 + HB],
                                asb[0:HD, :],
                                rec[:],
                            )
                        else:
                            tmp = tailp.tile([64, HB], bf16, tag="tmp")
                            nc.vector.tensor_mul(tmp[:], asb[0:HD, :], rec[:])
                            nc.sync.dma_start(
                                outT[64:128, hc, c0 : c0 + HB], tmp[:]
                            )

                    for h in range(NHC):
                        hp, hc = h % 2, h // 2
                        r0 = 64 * hp
                        acc = accp.tile([P, HB], f32, tag="acc")
                        sc_cur = emit_scores(0, hc, r0)
                        for kt in range(NKT):
                            at_t = attnp.tile([P, HB], bf16, tag="attn")
                            nc.scalar.activation(at_t[:], sc_cur[:], Exp)
                            if kt + 1 < NKT:
                                sc_cur = emit_scores(kt + 1, hc, r0)
                            if kt % 2 == 1:
                                s = next(fill_iter, None)
                                if s is not None:
                                    s()
                            emit_attnv(acc, h, kt, at_t)
                        emit_tail(acc, hp, hc)

                    # anything the fillers didn't get to runs here
                    drain(fill_iter)

                # output projection for q-half 1 drains at the end
                drain(oproj_steps(range(8, 16)))

    nc.compile()
    return nc


def _get_prog():
    if _PROG[0] is None:
        _PROG[0] = _build()
    return _PROG[0]


def make_in_maps(query, key, value, Wq, Wk, Wv, Wo):
    import ml_dtypes

    bf16 = ml_dtypes.bfloat16
    scale = np.float32(1.0 / np.sqrt(D))
    Wq_s = (np.asarray(Wq, np.float32) * scale).astype(bf16)
    Wk_s = np.asarray(Wk, np.float32).astype(bf16)
    Wv_s = np.asarray(Wv, np.float32).astype(bf16)
    Wo_s = np.asarray(Wo, np.float32).astype(bf16)
    in_maps = []
    for b in range(B):
        xqT = np.ascontiguousarray(np.asarray(query[b], np.float32).T.astype(bf16))
        xkT = np.ascontiguousarray(np.asarray(key[b], np.float32).T.astype(bf16))
        xvT = np.ascontiguousarray(np.asarray(value[b], np.float32).T.astype(bf16))
        for hh in range(2):
            sl = slice(hh * DH, (hh + 1) * DH)
            in_maps.append(
                {
                    "xq": xqT,
                    "xk": xkT,
                    "xv": xvT,
                    "wq": np.ascontiguousarray(Wq_s[:, sl]),
                    "wk": np.ascontiguousarray(Wk_s[:, sl]),
                    "wv": np.ascontiguousarray(Wv_s[:, sl]),
                    "wo": np.ascontiguousarray(Wo_s[sl, :]),
                }
            )
    return in_maps


def run(in_maps, trace=False, **kw):
    from concourse.bass_utils import run_bass_kernel_spmd

    nc = _get_prog()
    return run_bass_kernel_spmd(
        nc, in_maps, core_ids=list(range(NCORES)), trace=trace, **kw
    )


def kernel(query, key, value, Wq, Wk, Wv, Wo, bo):
    in_maps = make_in_maps(query, key, value, Wq, Wk, Wv, Wo)
    res = run(in_maps)
    bo = np.asarray(bo, np.float32)
    out = np.empty((B, S, D), np.float32)
    for b in range(B):
        out[b] = res.results[2 * b]["part"] + res.results[2 * b + 1]["part"] + bo
    return out
